# revision 22
# baseline (speedup 1.0000x reference)
"""CKA (RBF-kernel HSIC) on 8 Trainium2 NeuronCores — cyclic-symmetric v2.

Exploits K's symmetry with a circulant block schedule: the 4096 samples
form 32 row-blocks of 128; row-block m computes only the strip of columns
[128m, 128m + 17*128) cyclically (distances d = 0..16). Every unordered
block pair is covered once (d<16), the d=16 pair twice, the diagonal once,
so full-matrix sums are assembled as sum_m (2*P_m - D0_m - D16_m).
Row sums come from the ScalarE activation accumulator over each strip;
the lower-triangle contributions come from per-strip column sums
(ones^T @ E matmul over d in [1,15]) copied out of PSUM and un-rotated
on the host. This runs ~51% of the baseline's matmul/exp/product work.

Each core owns 4 tile-rows {2c, 2c+1, 30-2c, 31-2c} (equal strip columns
per core). SPMD uniformity: strips are addressed in canonical rolled
coordinates; the host pre-rolls two X^T windows per matrix per core so
every core runs the identical program on different data.

Device pipeline per strip (k-outer over the whole strip):
  3 fp8-DoubleRow passes + 1 bf16 aug pass (adds -|x_j|^2/2 via ones
  stationary) accumulate scaled Gram chunks into [128,1024] PSUM tiles;
  ScalarE exp's them into a bf16 E strip (accum_out = row sums); ones128
  matmuls produce replicated column sums whose row 0 is copied to SBUF
  (ScalarE for X strips, VectorE for Y) and DMA'd out; VectorE
  scalar_tensor_tensor accumulates the three Hadamard-product sums.
"""

import numpy as np
import ml_dtypes

BF16 = ml_dtypes.bfloat16
FP8 = ml_dtypes.float8_e4m3

N = 4096
D = 768
NCORES = 8
KC = 3                 # 256-row DoubleRow contraction blocks
NBLK = 32              # 128-row blocks
SPAN = 17 * 128        # strip width: distances 0..16
WIN = 18 * 128         # rolled window width (covers both rows of a half)

_cache = {}
LAST_RESULTS = None


def _strip_meta(r):
    """Canonical geometry for strip r (0..3). All offsets in columns."""
    s0 = 128 * (r % 2)            # strip start in the rolled window
    s1 = s0 + SPAN                # strip end
    segs = []                     # [lo, hi) aligned to the 1024 grid
    lo = s0
    while lo < s1:
        hi = min((lo // 1024 + 1) * 1024, s1)
        segs.append((lo, hi))
        lo = hi
    cs_lo, cs_hi = s0 + 128, s0 + 2048   # colsum span: d in [1, 15]
    return s0, s1, segs, cs_lo, cs_hi


def _chunks(lo, hi, step=512):
    out = []
    while lo < hi:
        nxt = min((lo // step + 1) * step, hi)
        out.append((lo, nxt))
        lo = nxt
    return out


def _build(inv_sigma_sq: float):
    import concourse.bacc as bacc
    import concourse.mybir as mybir
    import concourse.tile as tile

    fp32 = mybir.dt.float32
    bf16 = mybir.dt.bfloat16
    fp8 = mybir.dt.float8e4
    DRS = mybir.MatmulPerfMode.DoubleRowSwInterleave
    Exp = mybir.ActivationFunctionType.Exp
    mult = mybir.AluOpType.mult

    nc = bacc.Bacc(None)

    # ---- DRAM I/O ----
    mt_d = {}
    aug_d = {}
    for mat in "xy":
        for half in "ab":
            mt_d[mat, half] = nc.dram_tensor(
                f"mt_{mat}{half}", [KC, 128, 2, WIN], fp8, kind="ExternalInput")
            aug_d[mat, half] = nc.dram_tensor(
                f"aug_{mat}{half}", [2, WIN], bf16, kind="ExternalInput")
    bt_d = {m: nc.dram_tensor(f"bt_{m}", [4, KC, 128, 128, 2], fp8,
                              kind="ExternalInput") for m in "xy"}
    bias_d = {m: nc.dram_tensor(f"bias_{m}", [128, 4], fp32,
                                kind="ExternalInput") for m in "xy"}
    ones2_d = nc.dram_tensor("ones2", [2, 128], bf16, kind="ExternalInput")
    ones128_d = nc.dram_tensor("ones128", [128, 128], bf16,
                               kind="ExternalInput")

    racc_d = {m: nc.dram_tensor(f"racc_{m}", [128, 12], fp32,
                                kind="ExternalOutput") for m in "xy"}
    pacc_d = nc.dram_tensor("pacc", [128, 42], fp32, kind="ExternalOutput")
    cs_d = {m: nc.dram_tensor(f"cs_{m}", [4, WIN], fp32,
                              kind="ExternalOutput") for m in "xy"}

    with tile.TileContext(nc) as tc:
        with (
            tc.tile_pool(name="res", bufs=1) as res,
            tc.tile_pool(name="epool", bufs=2) as ep,
            tc.tile_pool(name="psum", bufs=1, space="PSUM") as pp,
        ):
            # ---- persistent SBUF ----
            mt_sb = {}
            aug_sb = {}
            for mat in "xy":
                for half in "ab":
                    for k in range(KC):
                        mt_sb[mat, half, k] = res.tile(
                            [128, 2, WIN], fp8, tag=f"mt{mat}{half}{k}",
                            name=f"mt{mat}{half}{k}")
                    aug_sb[mat, half] = res.tile(
                        [2, WIN], bf16, tag=f"aug{mat}{half}",
                        name=f"aug{mat}{half}")
            bt_sb = {}
            for m in "xy":
                for r in range(4):
                    for k in range(KC):
                        bt_sb[m, r, k] = res.tile(
                            [128, 128, 2], fp8, tag=f"bt{m}{r}{k}",
                            name=f"bt{m}{r}{k}")
            bias_sb = {m: res.tile([128, 4], fp32, tag=f"bias{m}",
                                   name=f"bias{m}") for m in "xy"}
            ones2_sb = res.tile([2, 128], bf16, tag="ones2", name="ones2")
            ones128_sb = res.tile([128, 128], bf16, tag="ones128",
                                  name="ones128")
            racc_sb = {m: res.tile([128, 12], fp32, tag=f"racc{m}",
                                   name=f"racc{m}") for m in "xy"}
            pacc_sb = res.tile([128, 42], fp32, tag="pacc", name="pacc")
            scr = res.tile([128, WIN], bf16, tag="scr", name="scr")

            # ---- input DMAs on the sync queue in first-use order;
            # tiny constants on the scalar queue in parallel ----
            nc.scalar.dma_start(bias_sb["x"][:], bias_d["x"][:])
            nc.scalar.dma_start(bias_sb["y"][:], bias_d["y"][:])
            nc.scalar.dma_start(ones2_sb[:], ones2_d[:])
            nc.scalar.dma_start(ones128_sb[:], ones128_d[:])
            for mat in "xy":
                for half in "ab":
                    nc.scalar.dma_start(aug_sb[mat, half][:],
                                        aug_d[mat, half][:])
            for k in range(KC):
                nc.sync.dma_start(bt_sb["x", 0, k][:], bt_d["x"][0, k])
            for k in range(2):
                nc.sync.dma_start(mt_sb["x", "a", k][:], mt_d["x", "a"][k])
            nc.scalar.dma_start(mt_sb["x", "a", 2][:], mt_d["x", "a"][2])
            for k in range(KC):
                nc.sync.dma_start(bt_sb["y", 0, k][:], bt_d["y"][0, k])
            nc.sync.dma_start(mt_sb["y", "a", 0][:], mt_d["y", "a"][0])
            nc.scalar.dma_start(mt_sb["y", "a", 1][:], mt_d["y", "a"][1])
            nc.sync.dma_start(mt_sb["y", "a", 2][:], mt_d["y", "a"][2])
            for r in (1, 2):
                for mat in "xy":
                    for k in range(KC):
                        nc.sync.dma_start(bt_sb[mat, r, k][:],
                                          bt_d[mat][r, k])
            for mat in "xy":
                for k in range(KC):
                    nc.sync.dma_start(mt_sb[mat, "b", k][:],
                                      mt_d[mat, "b"][k])
            for mat in "xy":
                for k in range(KC):
                    nc.sync.dma_start(bt_sb[mat, 3, k][:], bt_d[mat][3, k])

            # strip processing order: (x,r) then (y,r)
            e_t = {}          # (mat, r) -> E strip tile
            cs_jobs = []      # deferred colsum MMs+copies for previous strip
            racc_col = {"x": 0, "y": 0}

            def emit_cs(n):
                for _ in range(min(n, len(cs_jobs))):
                    cs_jobs.pop(0)()

            KINDS = (("x", "x"), ("y", "y"), ("x", "y"))

            def emit_stt(r, kinds, split=False):
                """Hadamard-product sums for tile-row r (strip, d0, d16).

                split=True emits the strip sum as per-seg pieces (extra
                accumulator columns 36..) so the final row's products
                overlap its activations instead of trailing them.
                """
                s0 = 128 * (r % 2)
                s1 = s0 + SPAN
                if split:
                    strip_pieces = [(s0, 1024), (1024, 2048), (2048, s1)]
                else:
                    strip_pieces = [(s0, s1)]
                for ki in kinds:
                    ia, ib = KINDS[ki]
                    ta = e_t[ia, r]
                    tb = e_t[ib, r]
                    pieces = (strip_pieces
                              + [(s0, s0 + 128), (s0 + 2048, s1)])
                    for pi, (a, b) in enumerate(pieces):
                        if split:
                            if pi < 3:       # seg pieces -> overflow cols
                                col = 36 + (ki - 1) * 3 + pi
                            else:
                                col = r * 9 + ki * 3 + (pi - 2)
                        else:
                            col = r * 9 + ki * 3 + pi
                        nc.vector.scalar_tensor_tensor(
                            out=scr[:, a:b], in0=ta[:, a:b], scalar=1.0,
                            in1=tb[:, a:b], op0=mult, op1=mult,
                            accum_out=pacc_sb[:, col:col + 1])

            for r in range(4):
                half = "a" if r < 2 else "b"
                s0, s1, segs, cs_lo, cs_hi = _strip_meta(r)
                for mat in "xy":
                    et = ep.tile([128, WIN], bf16, tag=f"e{mat}",
                                 name=f"e{mat}{r}")
                    e_t[mat, r] = et
                    g_tiles = {}
                    for si, (lo, hi) in enumerate(segs):
                        g_tiles[si] = pp.tile([128, 1024], fp32, tag="g",
                                              name=f"g{mat}{r}{si}", bufs=3)

                    def g_pass(k, seg_list):
                        first = True
                        if k < KC:
                            lhsT = bt_sb[mat, r, k][:]
                            mv = mt_sb[mat, half, k]
                            for si in seg_list:
                                lo, hi = segs[si]
                                for (a, b) in _chunks(lo, hi):
                                    nc.tensor.matmul(
                                        g_tiles[si][:, a - lo:b - lo],
                                        lhsT, mv[:, :, a:b],
                                        start=(k == 0), stop=False,
                                        perf_mode=DRS)
                        else:  # aug pass closes the accumulation
                            for si in seg_list:
                                lo, hi = segs[si]
                                for (a, b) in _chunks(lo, hi):
                                    nc.tensor.matmul(
                                        g_tiles[si][:, a - lo:b - lo],
                                        ones2_sb[:, :],
                                        aug_sb[mat, half][:, a:b],
                                        start=False, stop=True)

                    def act_seg(si):
                        lo, hi = segs[si]
                        col = racc_col[mat]
                        racc_col[mat] += 1
                        nc.scalar.activation(
                            et[:, lo:hi],
                            g_tiles[si][:, 0:hi - lo],
                            Exp,
                            bias=bias_sb[mat][:, r:r + 1],
                            scale=inv_sigma_sq,
                            accum_out=racc_sb[mat][:, col:col + 1])

                    rest = list(range(1, len(segs)))
                    g_pass(0, [0] + rest)
                    emit_cs(3)
                    g_pass(1, [0] + rest)
                    emit_cs(3)
                    g_pass(2, [0])
                    g_pass(KC, [0])      # aug seg0 -> ACT seg0 starts early
                    act_seg(0)
                    g_pass(2, rest)
                    emit_cs(2)
                    g_pass(KC, rest)
                    for si in rest:
                        act_seg(si)

                    # colsum jobs (deferred into the next strip's passes)
                    cs_sb = ep.tile([1, WIN], fp32, tag=f"cssb{mat}",
                                    name=f"cssb{mat}{r}")

                    def make_job(mat, r, et, cs_sb, a, b):
                        def job():
                            w = b - a
                            cs_ps = pp.tile([128, 512], fp32, tag="cs",
                                            name=f"cs{mat}{r}{a}", bufs=2)
                            nc.tensor.matmul(
                                cs_ps[:, 0:w], ones128_sb[:], et[:, a:b],
                                start=True, stop=True)
                            if mat == "x" or r == 3:
                                nc.scalar.copy(cs_sb[0:1, a:b],
                                               cs_ps[0:1, 0:w])
                            else:
                                nc.vector.tensor_copy(cs_sb[0:1, a:b],
                                                      cs_ps[0:1, 0:w])
                        return job

                    # chunk the colsum span, seg-aligned to keep deps tight
                    for si, (lo, hi) in enumerate(segs):
                        a0, b0 = max(lo, cs_lo), min(hi, cs_hi)
                        for (a, b) in _chunks(a0, b0):
                            if b > a:
                                cs_jobs.append(
                                    make_job(mat, r, et, cs_sb, a, b))
                    # final DMA of this strip's colsums, after its copies
                    def make_dma(mat, r, cs_sb, cs_lo, cs_hi):
                        def job():
                            nc.sync.dma_start(
                                cs_d[mat][r:r + 1, cs_lo:cs_hi],
                                cs_sb[0:1, cs_lo:cs_hi])
                        return job
                    cs_jobs.append(make_dma(mat, r, cs_sb, cs_lo, cs_hi))

                    if mat == "x":
                        # products: xx for this row now; yy/xy for the
                        # previous row (keeps DVE fed behind the cs copies)
                        if r > 0:
                            emit_stt(r - 1, (1, 2))
                        emit_stt(r, (0,))

            emit_stt(3, (1, 2), split=True)
            emit_cs(len(cs_jobs))

            for m in "xy":
                nc.sync.dma_start(racc_d[m][:], racc_sb[m][:])
            nc.sync.dma_start(pacc_d[:, 0:27], pacc_sb[:, 0:27])
            nc.sync.dma_start(pacc_d[:, 27:42], pacc_sb[:, 27:42])

    if not nc.is_finalized():
        nc.finalize()
    return nc


def _prep_matrix(A, inv_sigma_sq):
    """fp8 cast, transpose, aug rows (hi/lo bf16 of -|a_j|^2/2), bias."""
    A8 = A.astype(FP8)
    Af = A8.astype(np.float32)
    d = (Af.astype(np.float64) ** 2).sum(axis=1)
    AT = np.ascontiguousarray(A8.T)                        # [D, N]
    half = (-0.5 * d).astype(np.float32)
    hi = half.astype(BF16)
    lo = (half - hi.astype(np.float32)).astype(BF16)
    aug = np.stack([hi.astype(np.float32), lo.astype(np.float32)]).astype(BF16)
    bias = (-0.5 * inv_sigma_sq * d).astype(np.float32)
    return AT, aug, bias


def _dr_layout(AT_slice):
    """[768, W] fp8 -> [KC, 128, 2, W] DoubleRow layout (row=256k+128c+p)."""
    W = AT_slice.shape[1]
    return np.ascontiguousarray(
        AT_slice.reshape(KC, 2, 128, W).transpose(0, 2, 1, 3))


def _swi_layout(AT_slice):
    """[768, 128] fp8 -> [KC, 128, 128, 2] DoubleRowSwInterleave weights:
    per partition, column-reversed A/B interleaved pairs."""
    blk = _dr_layout(AT_slice)            # [KC, 128(p), 2(c), 128(j)]
    return np.ascontiguousarray(blk.transpose(0, 1, 3, 2)[:, :, ::-1, :])


def _own(c):
    return [2 * c, 2 * c + 1, 30 - 2 * c, 31 - 2 * c]


def _make_in_maps(X, Y, inv_sigma_sq):
    XT, xaug, xbias = _prep_matrix(X, inv_sigma_sq)
    YT, yaug, ybias = _prep_matrix(Y, inv_sigma_sq)
    prep = {"x": (XT, xaug, xbias), "y": (YT, yaug, ybias)}
    ones2 = np.ones((2, 128), dtype=BF16)
    ones128 = np.ones((128, 128), dtype=BF16)

    in_maps = []
    for c in range(NCORES):
        im = {"ones2": ones2, "ones128": ones128}
        rows = _own(c)
        for mat in "xy":
            AT, aug, bias = prep[mat]
            for half, base_blk in (("a", 2 * c), ("b", 30 - 2 * c)):
                idx = (128 * base_blk + np.arange(WIN)) % N
                im[f"mt_{mat}{half}"] = _dr_layout(AT[:, idx])
                im[f"aug_{mat}{half}"] = np.ascontiguousarray(aug[:, idx])
            im[f"bt_{mat}"] = np.ascontiguousarray(
                np.stack([_swi_layout(AT[:, 128 * m:128 * (m + 1)])
                          for m in rows]))
            im[f"bias_{mat}"] = np.ascontiguousarray(
                np.stack([bias[128 * m:128 * (m + 1)] for m in rows], axis=1))
        in_maps.append(im)
    return in_maps


def _combine(out):
    rvec = {"x": np.zeros(N, dtype=np.float64),
            "y": np.zeros(N, dtype=np.float64)}
    S = np.zeros(3, dtype=np.float64)           # xx, yy, xy
    for c in range(NCORES):
        res = out[c]
        rows = _own(c)
        for r, m in enumerate(rows):
            s0 = 128 * (r % 2)
            base = 128 * (2 * c) if r < 2 else 128 * (30 - 2 * c)
            for mat in "xy":
                racc = res[f"racc_{mat}"].astype(np.float64)
                rvec[mat][128 * m:128 * (m + 1)] += \
                    racc[:, 3 * r:3 * r + 3].sum(axis=1)
                cs = res[f"cs_{mat}"].astype(np.float64)
                lo, hi = s0 + 128, s0 + 2048
                absj = (base + np.arange(lo, hi)) % N
                np.add.at(rvec[mat], absj, cs[r, lo:hi])
            pacc = res["pacc"].astype(np.float64)
            for ki in range(3):
                b = r * 9 + ki * 3
                if r == 3 and ki in (1, 2):
                    strip = pacc[:, 36 + (ki - 1) * 3:39 + (ki - 1) * 3]\
                        .sum(axis=1)
                else:
                    strip = pacc[:, b]
                S[ki] += (2 * strip - pacc[:, b + 1]
                          - pacc[:, b + 2]).sum()

    n = float(N)
    rx, ry = rvec["x"], rvec["y"]
    tx, ty = rx.sum(), ry.sum()
    hsic_xx = S[0] - 2.0 / n * np.dot(rx, rx) + tx * tx / (n * n)
    hsic_yy = S[1] - 2.0 / n * np.dot(ry, ry) + ty * ty / (n * n)
    hsic_xy = S[2] - 2.0 / n * np.dot(rx, ry) + tx * ty / (n * n)
    return np.float32(hsic_xy / np.sqrt(hsic_xx * hsic_yy))


def kernel(X, Y, sigma):
    from concourse.bass_utils import run_bass_kernel_spmd

    X = np.asarray(X, dtype=np.float32)
    Y = np.asarray(Y, dtype=np.float32)
    sig = float(np.asarray(sigma))
    inv_sigma_sq = 1.0 / (sig * sig)

    if inv_sigma_sq not in _cache:
        _cache[inv_sigma_sq] = _build(inv_sigma_sq)
    nc = _cache[inv_sigma_sq]

    in_maps = _make_in_maps(X, Y, inv_sigma_sq)
    res = run_bass_kernel_spmd(nc, in_maps, list(range(NCORES)))
    global LAST_RESULTS
    LAST_RESULTS = res
    return _combine(res.results)


# revision 23
# speedup vs baseline: 1.1936x; 1.1936x over previous
"""CKA (RBF-kernel HSIC) on 8 Trainium2 NeuronCores — cyclic-symmetric v2.

Exploits K's symmetry with a circulant block schedule: the 4096 samples
form 32 row-blocks of 128; row-block m computes only the strip of columns
[128m, 128m + 17*128) cyclically (distances d = 0..16). Every unordered
block pair is covered once (d<16), the d=16 pair twice, the diagonal once,
so full-matrix sums are assembled as sum_m (2*P_m - D0_m - D16_m).
Row sums come from the ScalarE activation accumulator over each strip;
the lower-triangle contributions come from per-strip column sums
(ones^T @ E matmul over d in [1,15]) copied out of PSUM and un-rotated
on the host. This runs ~51% of the baseline's matmul/exp/product work.

Each core owns 4 tile-rows {2c, 2c+1, 30-2c, 31-2c} (equal strip columns
per core). SPMD uniformity: strips are addressed in canonical rolled
coordinates; the host pre-rolls two X^T windows per matrix per core so
every core runs the identical program on different data.

Device pipeline per strip (k-outer over the whole strip):
  3 fp8-DoubleRow passes + 1 bf16 aug pass (adds -|x_j|^2/2 via ones
  stationary) accumulate scaled Gram chunks into [128,1024] PSUM tiles;
  ScalarE exp's them into a bf16 E strip (accum_out = row sums); ones128
  matmuls produce replicated column sums whose row 0 is copied to SBUF
  (ScalarE for X strips, VectorE for Y) and DMA'd out; VectorE
  scalar_tensor_tensor accumulates the three Hadamard-product sums.
"""

import numpy as np
import ml_dtypes

BF16 = ml_dtypes.bfloat16
FP8 = ml_dtypes.float8_e4m3

N = 4096
D = 768
NCORES = 8
KC = 3                 # 256-row DoubleRow contraction blocks
NBLK = 32              # 128-row blocks
SPAN = 17 * 128        # strip width: distances 0..16
WIN = 18 * 128         # rolled window width (covers both rows of a half)

_cache = {}
LAST_RESULTS = None


def _strip_meta(r):
    """Canonical geometry for strip r (0..3). All offsets in columns."""
    s0 = 128 * (r % 2)            # strip start in the rolled window
    s1 = s0 + SPAN                # strip end
    segs = []                     # [lo, hi) aligned to the 1024 grid
    lo = s0
    while lo < s1:
        hi = min((lo // 1024 + 1) * 1024, s1)
        segs.append((lo, hi))
        lo = hi
    cs_lo, cs_hi = s0 + 128, s0 + 2048   # colsum span: d in [1, 15]
    return s0, s1, segs, cs_lo, cs_hi


def _chunks(lo, hi, step=512):
    out = []
    while lo < hi:
        nxt = min((lo // step + 1) * step, hi)
        out.append((lo, nxt))
        lo = nxt
    return out


def _build(inv_sigma_sq: float):
    import concourse.bacc as bacc
    import concourse.mybir as mybir
    import concourse.tile as tile

    fp32 = mybir.dt.float32
    bf16 = mybir.dt.bfloat16
    fp8 = mybir.dt.float8e4
    DRS = mybir.MatmulPerfMode.DoubleRowSwInterleave
    Exp = mybir.ActivationFunctionType.Exp
    mult = mybir.AluOpType.mult

    nc = bacc.Bacc(None)

    # ---- DRAM I/O ----
    mt_d = {}
    aug_d = {}
    for mat in "xy":
        for half in "ab":
            mt_d[mat, half] = nc.dram_tensor(
                f"mt_{mat}{half}", [KC, 128, 2, WIN], fp8, kind="ExternalInput")
            aug_d[mat, half] = nc.dram_tensor(
                f"aug_{mat}{half}", [2, WIN], bf16, kind="ExternalInput")
    bt_d = {m: nc.dram_tensor(f"bt_{m}", [4, KC, 128, 128, 2], fp8,
                              kind="ExternalInput") for m in "xy"}
    bias_d = {m: nc.dram_tensor(f"bias_{m}", [128, 4], fp32,
                                kind="ExternalInput") for m in "xy"}
    ones2_d = nc.dram_tensor("ones2", [2, 128], bf16, kind="ExternalInput")
    ones128_d = nc.dram_tensor("ones128", [128, 128], bf16,
                               kind="ExternalInput")

    racc_d = {m: nc.dram_tensor(f"racc_{m}", [128, 12], fp32,
                                kind="ExternalOutput") for m in "xy"}
    pacc_d = nc.dram_tensor("pacc", [128, 42], fp32, kind="ExternalOutput")
    cs_d = {m: nc.dram_tensor(f"cs_{m}", [4, WIN], fp32,
                              kind="ExternalOutput") for m in "xy"}

    with tile.TileContext(nc) as tc:
        with (
            tc.tile_pool(name="res", bufs=1) as res,
            tc.tile_pool(name="epool", bufs=2) as ep,
            tc.tile_pool(name="psum", bufs=1, space="PSUM") as pp,
        ):
            # ---- persistent SBUF ----
            mt_sb = {}
            aug_sb = {}
            for mat in "xy":
                for half in "ab":
                    for k in range(KC):
                        mt_sb[mat, half, k] = res.tile(
                            [128, 2, WIN], fp8, tag=f"mt{mat}{half}{k}",
                            name=f"mt{mat}{half}{k}")
                    aug_sb[mat, half] = res.tile(
                        [2, WIN], bf16, tag=f"aug{mat}{half}",
                        name=f"aug{mat}{half}")
            bt_sb = {}
            for m in "xy":
                for r in range(4):
                    for k in range(KC):
                        bt_sb[m, r, k] = res.tile(
                            [128, 128, 2], fp8, tag=f"bt{m}{r}{k}",
                            name=f"bt{m}{r}{k}")
            bias_sb = {m: res.tile([128, 4], fp32, tag=f"bias{m}",
                                   name=f"bias{m}") for m in "xy"}
            ones2_sb = res.tile([2, 128], bf16, tag="ones2", name="ones2")
            ones128_sb = res.tile([128, 128], bf16, tag="ones128",
                                  name="ones128")
            racc_sb = {m: res.tile([128, 12], fp32, tag=f"racc{m}",
                                   name=f"racc{m}") for m in "xy"}
            pacc_sb = res.tile([128, 42], fp32, tag="pacc", name="pacc")
            scr = res.tile([128, WIN], bf16, tag="scr", name="scr")

            # ---- input DMAs on the sync queue in first-use order;
            # tiny constants on the scalar queue in parallel ----
            nc.scalar.dma_start(bias_sb["x"][:], bias_d["x"][:])
            nc.scalar.dma_start(bias_sb["y"][:], bias_d["y"][:])
            nc.scalar.dma_start(ones2_sb[:], ones2_d[:])
            nc.scalar.dma_start(ones128_sb[:], ones128_d[:])
            for mat in "xy":
                for half in "ab":
                    nc.scalar.dma_start(aug_sb[mat, half][:],
                                        aug_d[mat, half][:])
            for k in range(KC):
                nc.sync.dma_start(bt_sb["x", 0, k][:], bt_d["x"][0, k])
            for k in range(KC):
                nc.sync.dma_start(mt_sb["x", "a", k][:], mt_d["x", "a"][k])
            for k in range(KC):
                nc.sync.dma_start(bt_sb["y", 0, k][:], bt_d["y"][0, k])
            for k in range(KC):
                nc.sync.dma_start(mt_sb["y", "a", k][:], mt_d["y", "a"][k])
            for r in (1, 2):
                for mat in "xy":
                    for k in range(KC):
                        nc.sync.dma_start(bt_sb[mat, r, k][:],
                                          bt_d[mat][r, k])
            for mat in "xy":
                for k in range(KC):
                    nc.sync.dma_start(mt_sb[mat, "b", k][:],
                                      mt_d[mat, "b"][k])
            for mat in "xy":
                for k in range(KC):
                    nc.sync.dma_start(bt_sb[mat, 3, k][:], bt_d[mat][3, k])

            # strip processing order: (x,r) then (y,r)
            e_t = {}          # (mat, r) -> E strip tile
            cs_jobs = []      # deferred colsum MMs+copies for previous strip
            racc_col = {"x": 0, "y": 0}

            def emit_cs(n):
                for _ in range(min(n, len(cs_jobs))):
                    cs_jobs.pop(0)()

            KINDS = (("x", "x"), ("y", "y"), ("x", "y"))

            def emit_stt(r, kinds, split=False):
                """Hadamard-product sums for tile-row r (strip, d0, d16).

                split=True emits the strip sum as per-seg pieces (extra
                accumulator columns 36..) so the final row's products
                overlap its activations instead of trailing them.
                """
                s0 = 128 * (r % 2)
                s1 = s0 + SPAN
                if split:
                    strip_pieces = [(s0, 1024), (1024, 2048), (2048, s1)]
                else:
                    strip_pieces = [(s0, s1)]
                for ki in kinds:
                    ia, ib = KINDS[ki]
                    ta = e_t[ia, r]
                    tb = e_t[ib, r]
                    pieces = (strip_pieces
                              + [(s0, s0 + 128), (s0 + 2048, s1)])
                    for pi, (a, b) in enumerate(pieces):
                        if split:
                            if pi < 3:       # seg pieces -> overflow cols
                                col = 36 + (ki - 1) * 3 + pi
                            else:
                                col = r * 9 + ki * 3 + (pi - 2)
                        else:
                            col = r * 9 + ki * 3 + pi
                        nc.vector.scalar_tensor_tensor(
                            out=scr[:, a:b], in0=ta[:, a:b], scalar=1.0,
                            in1=tb[:, a:b], op0=mult, op1=mult,
                            accum_out=pacc_sb[:, col:col + 1])

            for r in range(4):
                half = "a" if r < 2 else "b"
                s0, s1, segs, cs_lo, cs_hi = _strip_meta(r)
                for mat in "xy":
                    et = ep.tile([128, WIN], bf16, tag=f"e{mat}",
                                 name=f"e{mat}{r}")
                    e_t[mat, r] = et
                    g_tiles = {}
                    for si, (lo, hi) in enumerate(segs):
                        g_tiles[si] = pp.tile([128, 1024], fp32, tag="g",
                                              name=f"g{mat}{r}{si}", bufs=3)

                    def g_pass(k, seg_list):
                        first = True
                        if k < KC:
                            lhsT = bt_sb[mat, r, k][:]
                            mv = mt_sb[mat, half, k]
                            for si in seg_list:
                                lo, hi = segs[si]
                                for (a, b) in _chunks(lo, hi):
                                    nc.tensor.matmul(
                                        g_tiles[si][:, a - lo:b - lo],
                                        lhsT, mv[:, :, a:b],
                                        start=(k == 0), stop=False,
                                        perf_mode=DRS)
                        else:  # aug pass closes the accumulation
                            for si in seg_list:
                                lo, hi = segs[si]
                                for (a, b) in _chunks(lo, hi):
                                    nc.tensor.matmul(
                                        g_tiles[si][:, a - lo:b - lo],
                                        ones2_sb[:, :],
                                        aug_sb[mat, half][:, a:b],
                                        start=False, stop=True)

                    def act_seg(si):
                        lo, hi = segs[si]
                        col = racc_col[mat]
                        racc_col[mat] += 1
                        nc.scalar.activation(
                            et[:, lo:hi],
                            g_tiles[si][:, 0:hi - lo],
                            Exp,
                            bias=bias_sb[mat][:, r:r + 1],
                            scale=inv_sigma_sq,
                            accum_out=racc_sb[mat][:, col:col + 1])

                    rest = list(range(1, len(segs)))
                    g_pass(0, [0] + rest)
                    emit_cs(3)
                    g_pass(1, [0] + rest)
                    emit_cs(3)
                    g_pass(2, [0])
                    g_pass(KC, [0])      # aug seg0 -> ACT seg0 starts early
                    act_seg(0)
                    g_pass(2, rest)
                    emit_cs(2)
                    g_pass(KC, rest)
                    for si in rest:
                        act_seg(si)

                    # colsum jobs (deferred into the next strip's passes)
                    cs_sb = ep.tile([1, WIN], fp32, tag=f"cssb{mat}",
                                    name=f"cssb{mat}{r}")

                    def make_job(mat, r, et, cs_sb, a, b):
                        def job():
                            w = b - a
                            cs_ps = pp.tile([128, 512], fp32, tag="cs",
                                            name=f"cs{mat}{r}{a}", bufs=2)
                            nc.tensor.matmul(
                                cs_ps[:, 0:w], ones128_sb[:], et[:, a:b],
                                start=True, stop=True)
                            if mat == "x" or r == 3:
                                nc.scalar.copy(cs_sb[0:1, a:b],
                                               cs_ps[0:1, 0:w])
                            else:
                                nc.vector.tensor_copy(cs_sb[0:1, a:b],
                                                      cs_ps[0:1, 0:w])
                        return job

                    # chunk the colsum span, seg-aligned to keep deps tight
                    for si, (lo, hi) in enumerate(segs):
                        a0, b0 = max(lo, cs_lo), min(hi, cs_hi)
                        for (a, b) in _chunks(a0, b0):
                            if b > a:
                                cs_jobs.append(
                                    make_job(mat, r, et, cs_sb, a, b))
                    # final DMA of this strip's colsums, after its copies
                    def make_dma(mat, r, cs_sb, cs_lo, cs_hi):
                        def job():
                            nc.sync.dma_start(
                                cs_d[mat][r:r + 1, cs_lo:cs_hi],
                                cs_sb[0:1, cs_lo:cs_hi])
                        return job
                    cs_jobs.append(make_dma(mat, r, cs_sb, cs_lo, cs_hi))

                    if mat == "x":
                        # products: xx for this row now; yy/xy for the
                        # previous row (keeps DVE fed behind the cs copies)
                        if r > 0:
                            emit_stt(r - 1, (1, 2))
                        emit_stt(r, (0,))

            emit_stt(3, (1, 2), split=True)
            emit_cs(len(cs_jobs))

            for m in "xy":
                nc.sync.dma_start(racc_d[m][:], racc_sb[m][:])
            nc.sync.dma_start(pacc_d[:, 0:27], pacc_sb[:, 0:27])
            nc.sync.dma_start(pacc_d[:, 27:42], pacc_sb[:, 27:42])

    if not nc.is_finalized():
        nc.finalize()
    return nc


def _prep_matrix(A, inv_sigma_sq):
    """fp8 cast, transpose, aug rows (hi/lo bf16 of -|a_j|^2/2), bias."""
    A8 = A.astype(FP8)
    Af = A8.astype(np.float32)
    d = (Af.astype(np.float64) ** 2).sum(axis=1)
    AT = np.ascontiguousarray(A8.T)                        # [D, N]
    half = (-0.5 * d).astype(np.float32)
    hi = half.astype(BF16)
    lo = (half - hi.astype(np.float32)).astype(BF16)
    aug = np.stack([hi.astype(np.float32), lo.astype(np.float32)]).astype(BF16)
    bias = (-0.5 * inv_sigma_sq * d).astype(np.float32)
    return AT, aug, bias


def _dr_layout(AT_slice):
    """[768, W] fp8 -> [KC, 128, 2, W] DoubleRow layout (row=256k+128c+p)."""
    W = AT_slice.shape[1]
    return np.ascontiguousarray(
        AT_slice.reshape(KC, 2, 128, W).transpose(0, 2, 1, 3))


def _swi_layout(AT_slice):
    """[768, 128] fp8 -> [KC, 128, 128, 2] DoubleRowSwInterleave weights:
    per partition, column-reversed A/B interleaved pairs."""
    blk = _dr_layout(AT_slice)            # [KC, 128(p), 2(c), 128(j)]
    return np.ascontiguousarray(blk.transpose(0, 1, 3, 2)[:, :, ::-1, :])


def _own(c):
    return [2 * c, 2 * c + 1, 30 - 2 * c, 31 - 2 * c]


def _make_in_maps(X, Y, inv_sigma_sq):
    XT, xaug, xbias = _prep_matrix(X, inv_sigma_sq)
    YT, yaug, ybias = _prep_matrix(Y, inv_sigma_sq)
    prep = {"x": (XT, xaug, xbias), "y": (YT, yaug, ybias)}
    ones2 = np.ones((2, 128), dtype=BF16)
    ones128 = np.ones((128, 128), dtype=BF16)

    in_maps = []
    for c in range(NCORES):
        im = {"ones2": ones2, "ones128": ones128}
        rows = _own(c)
        for mat in "xy":
            AT, aug, bias = prep[mat]
            for half, base_blk in (("a", 2 * c), ("b", 30 - 2 * c)):
                idx = (128 * base_blk + np.arange(WIN)) % N
                im[f"mt_{mat}{half}"] = _dr_layout(AT[:, idx])
                im[f"aug_{mat}{half}"] = np.ascontiguousarray(aug[:, idx])
            im[f"bt_{mat}"] = np.ascontiguousarray(
                np.stack([_swi_layout(AT[:, 128 * m:128 * (m + 1)])
                          for m in rows]))
            im[f"bias_{mat}"] = np.ascontiguousarray(
                np.stack([bias[128 * m:128 * (m + 1)] for m in rows], axis=1))
        in_maps.append(im)
    return in_maps


def _combine(out):
    rvec = {"x": np.zeros(N, dtype=np.float64),
            "y": np.zeros(N, dtype=np.float64)}
    S = np.zeros(3, dtype=np.float64)           # xx, yy, xy
    for c in range(NCORES):
        res = out[c]
        rows = _own(c)
        for r, m in enumerate(rows):
            s0 = 128 * (r % 2)
            base = 128 * (2 * c) if r < 2 else 128 * (30 - 2 * c)
            for mat in "xy":
                racc = res[f"racc_{mat}"].astype(np.float64)
                rvec[mat][128 * m:128 * (m + 1)] += \
                    racc[:, 3 * r:3 * r + 3].sum(axis=1)
                cs = res[f"cs_{mat}"].astype(np.float64)
                lo, hi = s0 + 128, s0 + 2048
                absj = (base + np.arange(lo, hi)) % N
                np.add.at(rvec[mat], absj, cs[r, lo:hi])
            pacc = res["pacc"].astype(np.float64)
            for ki in range(3):
                b = r * 9 + ki * 3
                if r == 3 and ki in (1, 2):
                    strip = pacc[:, 36 + (ki - 1) * 3:39 + (ki - 1) * 3]\
                        .sum(axis=1)
                else:
                    strip = pacc[:, b]
                S[ki] += (2 * strip - pacc[:, b + 1]
                          - pacc[:, b + 2]).sum()

    n = float(N)
    rx, ry = rvec["x"], rvec["y"]
    tx, ty = rx.sum(), ry.sum()
    hsic_xx = S[0] - 2.0 / n * np.dot(rx, rx) + tx * tx / (n * n)
    hsic_yy = S[1] - 2.0 / n * np.dot(ry, ry) + ty * ty / (n * n)
    hsic_xy = S[2] - 2.0 / n * np.dot(rx, ry) + tx * ty / (n * n)
    return np.float32(hsic_xy / np.sqrt(hsic_xx * hsic_yy))


def kernel(X, Y, sigma):
    from concourse.bass_utils import run_bass_kernel_spmd

    X = np.asarray(X, dtype=np.float32)
    Y = np.asarray(Y, dtype=np.float32)
    sig = float(np.asarray(sigma))
    inv_sigma_sq = 1.0 / (sig * sig)

    if inv_sigma_sq not in _cache:
        _cache[inv_sigma_sq] = _build(inv_sigma_sq)
    nc = _cache[inv_sigma_sq]

    in_maps = _make_in_maps(X, Y, inv_sigma_sq)
    res = run_bass_kernel_spmd(nc, in_maps, list(range(NCORES)))
    global LAST_RESULTS
    LAST_RESULTS = res
    return _combine(res.results)
